# revision 44
# baseline (speedup 1.0000x reference)
"""GQA attention kernel for 8 trn2 NeuronCores (tensor-parallel over heads).

Problem: B=1, S=2048, D=2048, NQ=32 q heads, NKV=8 kv heads, HD=64.
Core i handles q heads 4i..4i+3 and kv head i; out = sum of per-core partials.

v3: fp16 matmul pipeline; x pre-transposed on host; K-head duplication baked
into Wk.  ACT-table thrash eliminated (HAM throttle fix): phase 1 uses a
single Abs_reciprocal_sqrt activation for 1/rms, phase 3 uses only Exp;
softmax denominators inverted with the custom-DVE reciprocal_approx_fast.
Epilogue (normalize + out-projection) is software-pipelined one step behind
the attention loop so the PE stream never has multi-us gaps.  exp batched
over [128,1024] PSUM pairs; pair causal masks from two host tables, one
applied on GpSimd, one on DVE.

Layout (all seq-transposed; zero device transposes):
  xT  [128, 4(sc), 16(kc), 512] fp16 from host
  Q^T [128 = 2 heads x 64, S] per head pair,  K^T [128 = kv head x2, S]
  V   [S, 64+1] fp16 with ones column (softmax sums fall out of PV matmul)
  S^T pair = K^T_slices.T @ Q^T (2 blocks) -> one exp -> PV: V_ext.T @ expS^T
  row 64 of PV psum = denominators; rec16 = 4096 * approx(1/d) fp16 with the
  2^-12 folded into the broadcast-matmul ones vector (fp16 range safety)
  out-proj: lhsT = O^T directly, partial written to DRAM as fp16
"""

import os
import sys

sys.path.insert(0, "/opt/trn_rl_repo")

import numpy as np

S = 2048
D = 2048
HD = 64
NQ = 32
NKV = 8
P = 128
EPS = 1e-6
SCALE = 0.125  # 1/sqrt(HD)
N_CORES = 8
OUT_DESCALE = 2.0 ** -4  # device writes 16*out (fp16 range management)

_CACHE = {}
LAST_RESULTS = None


def _build_nc():
    import concourse.bass as bass
    import concourse.tile as tile
    from concourse import bacc, mybir

    f16 = mybir.dt.float16
    f32 = mybir.dt.float32
    nc = bacc.Bacc("TRN2", target_bir_lowering=False, debug=False)

    def dram_in(name, shape, dt=f16):
        return nc.dram_tensor(name, list(shape), dt, kind="ExternalInput").ap()

    io = {
        "xt": dram_in("xt", (P, 4, 16, 512)),
        "wq": dram_in("wq", (P, 16, 256)),
        "wk2": dram_in("wk2", (P, 16, P)),
        "wv": dram_in("wv", (P, 16, HD)),
        "wo": dram_in("wo", (P, 2, D)),
        "cos4": dram_in("cos4", (P, S)),
        "sin4s": dram_in("sin4s", (P, S)),
        "rot2": dram_in("rot2", (P, P)),
        "sel2": dram_in("sel2", (2, P)),
        "ones2": dram_in("ones2", (P, 2)),
        "onesd": dram_in("onesd", (1, HD)),
        "ident64": dram_in("ident64", (HD, HD)),
        "maskA": dram_in("maskA", (P, 1024)),
        "maskB": dram_in("maskB", (P, 1024)),
        "gq2": dram_in("gq2", (P, 1), f32),
        "gk2": dram_in("gk2", (P, 1), f32),
        "out": nc.dram_tensor("out", [S, D], f16, kind="ExternalOutput").ap(),
    }

    from contextlib import ExitStack

    with tile.TileContext(nc) as tc, ExitStack() as ctx:
        _emit(ctx, tc, io, bass, mybir)
    nc.compile()
    return nc


def _emit(ctx, tc, io, bass, mybir):
    nc = tc.nc
    f16 = mybir.dt.float16
    f32 = mybir.dt.float32
    Exp = mybir.ActivationFunctionType.Exp
    Sqrt = mybir.ActivationFunctionType.Sqrt
    mult = mybir.AluOpType.mult

    cpool = ctx.enter_context(tc.tile_pool(name="consts", bufs=1))
    pers = ctx.enter_context(tc.tile_pool(name="persist", bufs=1))

    def cload(name, shape, dt=f16, n_dma=1, eng=None):
        t = cpool.tile(list(shape), dt, tag=name, name=name)
        e = eng or nc.sync
        if n_dma == 1:
            e.dma_start(t[:], io[name][:])
        else:
            for c in range(n_dma):
                e.dma_start(t[:, c], io[name][:, c])
        return t

    # phase-1 weights on the scalar DMA queue, xt on sync: the two streams
    # overlap so the first projection starts ~4us earlier
    wq = cload("wq", (P, 16, 256), eng=nc.scalar)
    wk2 = cload("wk2", (P, 16, P), eng=nc.scalar)
    wv = cload("wv", (P, 16, HD), eng=nc.scalar)
    xt = cload("xt", (P, 4, 16, 512), n_dma=4)
    cos4 = cload("cos4", (P, S))
    sin4s = cload("sin4s", (P, S))
    rot2 = cload("rot2", (P, P))
    sel2 = cload("sel2", (2, P))
    ones2 = cload("ones2", (P, 2))
    gq2 = cload("gq2", (P, 1), f32)
    gk2 = cload("gk2", (P, 1), f32)
    maskA = cload("maskA", (P, 1024))
    maskB = cload("maskB", (P, 1024))
    onesd = cload("onesd", (1, HD))
    ident64 = cload("ident64", (HD, HD))
    wo = cload("wo", (P, 2, D))

    # ---- persistent activations ----
    QTh = pers.tile([P, 2, S], f16, tag="qth")  # roped Q, head pairs
    KTh = pers.tile([P, S], f16, tag="kth")  # rows 64-127 duplicate 0-63
    V = pers.tile([P, 16, HD + 1], f16, tag="v")  # [seq128, kblock, hd+ones]
    OT = pers.tile([P, 2, S], f16, tag="ot")  # attn out transposed
    rstd = pers.tile([2, 3, S], f16, tag="rstd")  # 1/rms per (head, seq)

    # Power-of-2 range management so every fp16 intermediate stays normal:
    # ones col 2^-12 -> rec16 = 4096/d; onesd 2^-2 -> bcs = 1024/d;
    # OT = 1024*O; Wo host-scaled 2^-6 -> device out = 16*true out;
    # the host applies the final 2^-4.
    nc.vector.memset(V[:, :, HD : HD + 1], 2.0 ** -12)

    # SBUF pools shared by phase 1 and the deferred sc=3 work in phase 3
    rawp = ctx.enter_context(tc.tile_pool(name="raw", bufs=4))
    sqp = ctx.enter_context(tc.tile_pool(name="sq", bufs=2))
    vtsp = ctx.enter_context(tc.tile_pool(name="vts", bufs=2))
    nhp = ctx.enter_context(tc.tile_pool(name="nh", bufs=2))
    t12p = ctx.enter_context(tc.tile_pool(name="t12", bufs=4))

    def rope1(pi, g, dst, cs, raws, bct, swt):
        bc = bct()
        nc.tensor.matmul(
            bc, sel2[:, :], rstd[:, pi, cs], start=True, stop=True)
        nh = nhp.tile([P, 512], f16, tag="nh", name="nh")
        nc.vector.scalar_tensor_tensor(nh, raws[pi], g[:, :], bc, mult, mult)
        sw = swt()
        nc.tensor.matmul(sw, rot2[:, :], nh, start=True, stop=True)
        t1 = t12p.tile([P, 512], f16, tag="t1", name="t1")
        nc.vector.tensor_mul(t1, nh, cos4[:, cs])
        t2 = t12p.tile([P, 512], f16, tag="t2", name="t2")
        nc.vector.tensor_mul(t2, sw, sin4s[:, cs])
        nc.vector.tensor_add(dst, t1, t2)

    def rope_targets(cs):
        return [(gq2, QTh[:, 0, cs]), (gq2, QTh[:, 1, cs]), (gk2, KTh[:, cs])]

    # ============ Phase 1+2: projections + RMSNorm + RoPE, per seq chunk ====
    # Software-pipelined: the rope for chunk sc-1 is emitted inside chunk sc's
    # projection stream, so its rstd dependency chain (DVE copy -> approx ->
    # ACT Sqrt) is long since resolved and the PE never stalls on it.
    # Chunk sc=3 is only needed by q-chunk 3: its entire pipeline is deferred
    # into the attention phase as PE filler work.
    with (
        tc.tile_pool(name="ppsum", bufs=2, space="PSUM") as pp,
        tc.tile_pool(name="sspsum", bufs=2, space="PSUM") as ssp,
        tc.tile_pool(name="bcpsum", bufs=1, space="PSUM") as bcp,
        tc.tile_pool(name="swpsum", bufs=1, space="PSUM") as swp,
        tc.tile_pool(name="vtpsum", bufs=1, space="PSUM") as vtp,
        tc.tile_pool(name="vrpsum", bufs=1, space="PSUM") as vrp,
    ):
        rope_pend = []

        def rope(sc, raws):
            cs = slice(sc * 512, (sc + 1) * 512)
            for pi, (g, dst) in enumerate(rope_targets(cs)):
                rope1(pi, g, dst, cs, raws,
                      lambda: bcp.tile([P, 512], f32, tag="bc", name="bc"),
                      lambda: swp.tile([P, 512], f32, tag="sw", name="sw"))

        for sc in range(3):
            cs = slice(sc * 512, (sc + 1) * 512)
            xts = xt[:, sc]  # [P, 16, 512]
            raws = []
            sss = []
            for pi in range(3):  # Qa, Qb, K projections
                if pi == 0:
                    wsl = lambda kc: wq[:, kc, 0:128]
                elif pi == 1:
                    wsl = lambda kc: wq[:, kc, 128:256]
                else:
                    wsl = lambda kc: wk2[:, kc, :]
                ps = pp.tile([P, 512], f32, tag="p", name="ps")
                for kc in range(16):
                    nc.tensor.matmul(
                        ps, wsl(kc), xts[:, kc, :],
                        start=(kc == 0), stop=(kc == 15),
                    )
                raw = rawp.tile([P, 512], f16, tag="raw", name="raw", bufs=8)
                nc.vector.tensor_copy(raw, ps)
                sq = sqp.tile([P, 512], f16, tag="sq", name="sq")
                nc.vector.tensor_mul(sq, raw, raw)
                raws.append(raw)
                sss.append(sq)
            for pi in range(3):  # per-head sum of squares + 1/rms
                ssps = ssp.tile([2, 512], f32, tag="ss", name="ssps")
                nc.tensor.matmul(ssps, ones2[:, :], sss[pi], start=True, stop=True)
                # 1/std = sqrt(64 * approx(1/sumsq)); keeps phase 1 on the
                # Sqrt act table only (no Ln/Exp table thrash -> HAM warm).
                # approx_fast is a raw-bits trick: stage PSUM -> SBUF first.
                ssc = rawp.tile([2, 512], f32, tag="ssc", name="ssc")
                nc.vector.tensor_copy(ssc, ssps)
                r32 = rawp.tile([2, 512], f32, tag="r32s", name="r32s")
                nc.vector.reciprocal_approx_fast(out=r32[:, :], in_=ssc[:, :])
                nc.scalar.activation(rstd[:, pi, cs], r32, Sqrt, scale=float(HD))
            # V projection, transposed (M=512 keeps LDWEIGHTS off the
            # critical path), then flipped back by PE transposes.
            # sc=3's V is only needed by q-chunk 3: deferred into the
            # attention phase as PE filler for the filler-less first region.
            if sc < 3:
                vt = vtp.tile([HD, 512], f32, tag="vt", name="vt")
                for kc in range(16):
                    nc.tensor.matmul(
                        vt, wv[:, kc, :], xts[:, kc, :],
                        start=(kc == 0), stop=(kc == 15),
                    )
                vts = vtsp.tile([HD, 512], f16, tag="vts", name="vts")
                nc.vector.tensor_copy(vts, vt)
            if rope_pend:
                rope(*rope_pend.pop())  # rope for sc-1: deps long resolved
            if sc < 3:
                for ms in range(4):
                    vr = vrp.tile([P, HD], f16, tag="vr", name="vr")
                    nc.tensor.transpose(
                        vr[:], vts[:, ms * P : (ms + 1) * P], ident64[:, :])
                    nc.scalar.copy(V[:, sc * 4 + ms, 0:HD], vr[:])
            rope_pend.append((sc, raws))
        rope(*rope_pend.pop())
        # preload the Exp act table while phase-1 work drains
        dmy = rawp.tile([2, 16], f16, tag="dmy", name="dmy")
        nc.scalar.activation(dmy[:, :], rstd[:, 0, 0:16], Exp)

    # ================= Phase 3: attention + out-projection =================
    with (
        tc.tile_pool(name="exps", bufs=4) as ep,
        tc.tile_pool(name="rcp", bufs=2) as rcp,
        tc.tile_pool(name="stg", bufs=2) as stgp,
        tc.tile_pool(name="ov", bufs=3) as ovp,
        tc.tile_pool(name="spsum", bufs=2, space="PSUM") as sp,
        tc.tile_pool(name="opsum", bufs=2, space="PSUM") as op_,
        tc.tile_pool(name="bpsum", bufs=1, space="PSUM") as bp,
        tc.tile_pool(name="oppsum", bufs=1, space="PSUM") as opp,
    ):
        pending = []  # deferred epilogue emitters (PE ops of prior steps)

        def flush(n=None):
            cnt = len(pending) if n is None else min(n, len(pending))
            for _ in range(cnt):
                pending.pop(0)()

        # Chunk sc=3's entire pipeline, deferred: dense dependency-free PE
        # filler for the early attention regions (only q-chunk 3 reads it).
        cs3 = slice(3 * 512, 4 * 512)
        raws3 = [None] * 3
        sss3 = [None] * 3
        st3 = {}

        def wsl3(pi, kc):
            if pi == 0:
                return wq[:, kc, 0:128]
            if pi == 1:
                return wq[:, kc, 128:256]
            return wk2[:, kc, :]

        for pi in range(3):
            def p3a(pi=pi):
                ps = opp.tile([P, 512], f32, tag="op", name="ps3p")
                st3[pi] = ps
                for kc in range(8):
                    nc.tensor.matmul(
                        ps, wsl3(pi, kc), xt[:, 3, kc, :],
                        start=(kc == 0), stop=False)

            def p3b(pi=pi):
                ps = st3.pop(pi)
                for kc in range(8, 16):
                    nc.tensor.matmul(
                        ps, wsl3(pi, kc), xt[:, 3, kc, :],
                        start=False, stop=(kc == 15))
                raw = rawp.tile([P, 512], f16, tag="raw", name="raw", bufs=8)
                nc.vector.tensor_copy(raw, ps)
                sq = sqp.tile([P, 512], f16, tag="sq", name="sq")
                nc.vector.tensor_mul(sq, raw, raw)
                raws3[pi] = raw
                sss3[pi] = sq

            pending.append(p3a)
            pending.append(p3b)

        def ss3():
            for pi in range(3):
                ssps = bp.tile([HD, 512], f32, tag="b", name="ss3")[0:2]
                nc.tensor.matmul(
                    ssps, ones2[:, :], sss3[pi], start=True, stop=True)
                ssc = rawp.tile([2, 512], f32, tag="ssc", name="ssc")
                nc.vector.tensor_copy(ssc, ssps)
                r32 = rawp.tile([2, 512], f32, tag="r32s", name="r32s")
                nc.vector.reciprocal_approx_fast(out=r32[:, :], in_=ssc[:, :])
                nc.scalar.activation(rstd[:, pi, cs3], r32, Sqrt, scale=float(HD))

        pending.append(ss3)
        for ms in range(4):
            def vunit(ms=ms):
                pv = bp.tile([P, HD], f32, tag="b", name="pv")
                for kc in range(16):
                    nc.tensor.matmul(
                        pv, xt[:, 3, kc, ms * P : (ms + 1) * P], wv[:, kc, :],
                        start=(kc == 0), stop=(kc == 15))
                nc.scalar.copy(V[:, 12 + ms, 0:HD], pv)

            pending.append(vunit)

        for pi in range(3):
            def rope3(pi=pi):
                g = gq2 if pi < 2 else gk2
                dst = QTh[:, pi, cs3] if pi < 2 else KTh[:, cs3]
                rope1(pi, g, dst, cs3, raws3,
                      lambda: opp.tile([P, 512], f32, tag="op", name="bc3"),
                      lambda: opp.tile([P, 512], f32, tag="op", name="sw3"))

            pending.append(rope3)

        # qc order: start dense-ish, bury the sparse short chunks (qc 0/1) in
        # the middle, end on the densest chunk -- keeps PE utilization above
        # the HAM throttle threshold for most of the phase
        for qc in (2, 1, 0, 3):
            qs = slice(qc * 512, (qc + 1) * 512)
            for h in range(4):
                pair, poff = h // 2, (h % 2) * HD
                npair = 2 * qc + 2  # kb block pairs (kb = 2j, 2j+1)
                nkb = 2 * npair
                po = op_.tile([HD + 1, 512], f32, tag="o", name="po")
                Q = QTh[poff : poff + HD, pair, qs]

                def spair(j):
                    ps2 = sp.tile([P, 1024], f32, tag="s", name="ps2")
                    for u in range(2):
                        kb = 2 * j + u
                        nc.tensor.matmul(
                            ps2[:, u * 512 : (u + 1) * 512],
                            KTh[poff : poff + HD, kb * P : (kb + 1) * P],
                            Q, start=True, stop=True,
                        )
                    es2 = ep.tile([P, 1024], f16, tag="e", name="es2")
                    nc.scalar.activation(es2, ps2, Exp, scale=SCALE)
                    # gpsimd mask is slow (~2.1us); keep it off the critical
                    # path of the shortest q-chunk, use it to relieve DVE else
                    if j == 2 * qc:
                        eng = nc.gpsimd if qc >= 1 else nc.vector
                        eng.tensor_mul(es2, es2, maskA[:, :])
                    elif j == 2 * qc + 1:
                        nc.vector.tensor_mul(es2, es2, maskB[:, :])
                    return es2

                def ppair(j, es2):
                    for u in range(2):
                        kb = 2 * j + u
                        nc.tensor.matmul(
                            po, V[:, kb, :], es2[:, u * 512 : (u + 1) * 512],
                            start=(kb == 0), stop=(kb == nkb - 1),
                        )

                prev = spair(0)
                flush(2)
                for j in range(1, npair):
                    cur = spair(j)
                    ppair(j - 1, prev)
                    prev = cur
                    flush(2)
                ppair(npair - 1, prev)

                # denominator reciprocal now (DVE, deps ready soon); the
                # broadcast matmul + normalize trail into the next head.
                # V ones column holds 2^-12, so den = d*2^-12 and
                # rec = 4096/d -- comfortably inside fp16 normal range.
                den = rcp.tile([1, 512], f32, tag="den", name="den")
                nc.vector.tensor_copy(den, po[HD : HD + 1, :])
                rec32 = rcp.tile([1, 512], f32, tag="r32", name="rec32")
                nc.vector.reciprocal_approx_fast(out=rec32[:, :], in_=den[:, :])
                rec16 = rcp.tile([1, 512], f16, tag="r16", name="rec16")
                nc.vector.tensor_copy(rec16[:, :], rec32[:, :])

                def normalize(po=po, pair=pair, poff=poff, qs=qs, rec16=rec16):
                    bcd = bp.tile([HD, 512], f32, tag="b", name="bcd")
                    nc.tensor.matmul(bcd, onesd[:, :], rec16[:, :], start=True, stop=True)
                    bcs = stgp.tile([HD, 512], f16, tag="bcs", name="bcs")
                    nc.vector.tensor_copy(bcs, bcd)
                    if poff == 0:
                        nc.vector.tensor_mul(OT[0:HD, pair, qs], po[0:HD, :], bcs)
                    else:
                        stg = stgp.tile([HD, 512], f16, tag="stg", name="stg")
                        nc.vector.tensor_mul(stg, po[0:HD, :], bcs)
                        nc.sync.dma_start(OT[HD:P, pair, qs], stg[:])

                pending.append(normalize)

            # out-projection units for this q chunk, deferred into next qc
            for ms in range(4):
                for dc in range(4):
                    def outproj(ms=ms, dc=dc, qc=qc):
                        sl = slice(qc * 512 + ms * P, qc * 512 + (ms + 1) * P)
                        if qc == 3:
                            # final chunk's drain: borrow the score psum slots
                            # (idle by now) so the tail pipelines 3-deep
                            pso = sp.tile([P, 1024], f32, tag="s", name="pso2")[:, 0:512]
                        else:
                            pso = opp.tile([P, 512], f32, tag="op", name="pso")
                        for kc in range(2):
                            nc.tensor.matmul(
                                pso, OT[:, kc, sl],
                                wo[:, kc, dc * 512 : (dc + 1) * 512],
                                start=(kc == 0), stop=(kc == 1),
                            )
                        ov = ovp.tile([P, 512], f16, tag="ov", name="ov")
                        if dc % 2 == 0:  # split psum drain across ACT + DVE
                            nc.scalar.copy(ov[:], pso[:])
                        else:
                            nc.vector.tensor_copy(ov[:], pso[:])
                        nc.sync.dma_start(
                            io["out"][sl, dc * 512 : (dc + 1) * 512], ov[:])

                    pending.append(outproj)
        flush()


def _prep_core_inputs(i, x, cos, sin, g_q, g_k, Wq, Wk, Wv, Wo):
    c0 = i * 4 * HD
    k0 = i * HD
    x2d = x.reshape(S, D)
    # xt[p, sc, kc, j] = x[sc*512+j, kc*128+p]
    xt = np.ascontiguousarray(
        x2d.T.reshape(16, P, 4, 512).transpose(1, 2, 0, 3).astype(np.float16))
    wq = np.ascontiguousarray(
        Wq[:, c0 : c0 + 256].reshape(16, P, 256).transpose(1, 0, 2)
    ).astype(np.float16)
    wkd = np.concatenate([Wk[:, k0 : k0 + HD]] * 2, axis=1)  # dup kv head
    wk2 = np.ascontiguousarray(
        wkd.reshape(16, P, P).transpose(1, 0, 2)).astype(np.float16)
    wv = np.ascontiguousarray(
        Wv[:, k0 : k0 + HD].reshape(16, P, HD).transpose(1, 0, 2)
    ).astype(np.float16)
    wo = np.ascontiguousarray(
        Wo[c0 : c0 + 2 * P, :].reshape(2, P, D).transpose(1, 0, 2) * 2.0 ** -6
    ).astype(np.float16)
    cosT = cos.T.astype(np.float32)  # [32, S]
    sinT = sin.T.astype(np.float32)
    cos4 = np.tile(cosT, (4, 1)).astype(np.float16)
    sin4s = np.concatenate([-sinT, sinT, -sinT, sinT], axis=0).astype(np.float16)
    gq2 = np.tile(g_q, 2)[:, None].astype(np.float32)
    gk2 = np.tile(g_k, 2)[:, None].astype(np.float32)
    tri = np.triu(np.ones((P, P), dtype=np.float16))  # [k within blk, q]
    mask0 = np.concatenate([tri, np.ones((P, 384), dtype=np.float16)], axis=1)
    maskz = np.concatenate([np.zeros((P, 512), dtype=np.float16), mask0], axis=1)

    def mz(o):
        return maskz[:, 512 - o * P : 1024 - o * P]

    maskA = np.ascontiguousarray(np.concatenate([mz(0), mz(1)], axis=1))
    maskB = np.ascontiguousarray(np.concatenate([mz(2), mz(3)], axis=1))
    ones2 = np.zeros((P, 2), dtype=np.float16)
    ones2[:HD, 0] = 1.0
    ones2[HD:, 1] = 1.0
    sel2 = np.ascontiguousarray(ones2.T)
    r64 = np.roll(np.eye(HD, dtype=np.float16), 32, axis=0)
    rot2 = np.zeros((P, P), dtype=np.float16)
    rot2[:HD, :HD] = r64
    rot2[HD:, HD:] = r64
    return {
        "xt": xt, "wq": wq, "wk2": wk2, "wv": wv, "wo": wo,
        "cos4": np.ascontiguousarray(cos4),
        "sin4s": np.ascontiguousarray(sin4s),
        "gq2": gq2, "gk2": gk2, "maskA": maskA, "maskB": maskB,
        "ones2": ones2, "sel2": sel2,
        "onesd": np.full((1, HD), 2.0 ** -2, dtype=np.float16),
        "ident64": np.eye(HD, dtype=np.float16),
        "rot2": rot2,
    }


def kernel(x, cos, sin, g_q, g_k, Wq, Wk, Wv, Wo):
    global LAST_RESULTS
    from concourse.bass_utils import run_bass_kernel_spmd

    if "nc" not in _CACHE:
        _CACHE["nc"] = _build_nc()
    nc = _CACHE["nc"]

    args = [np.asarray(a, dtype=np.float32) for a in
            (x, cos, sin, g_q, g_k, Wq, Wk, Wv, Wo)]
    in_maps = [_prep_core_inputs(i, *args) for i in range(N_CORES)]
    trace = bool(os.environ.get("BASS_TRACE"))
    res = run_bass_kernel_spmd(nc, in_maps, list(range(N_CORES)), trace=trace)
    LAST_RESULTS = res
    out = np.zeros((S, D), dtype=np.float32)
    for r in res.results:
        out += r["out"].astype(np.float32)
    out *= OUT_DESCALE  # undo the device-side power-of-2 range scaling
    return out.reshape(1, S, D)
